# revision 1
# baseline (speedup 1.0000x reference)
"""Trainium2 Bass kernel for CrossAttention + residual + LayerNorm.

Problem: B=4, Sq=Skv=2048, D=512, H=8 heads (dh=64), fp32 I/O.

Sharding (8 cores, no collectives): core c handles batch b=c//2 and query-row
half r=c%2 (1024 q rows). Each core loads its x slice [1024,512], the full
cross-attn input for its batch [2048,512], and all weights; computes
q/k/v projections, per-head softmax(q k^T / 8) @ v, output projection,
residual add and layernorm for its 1024 rows. Host reassembles.

Layout strategy (all matmul operands bf16, fp32 PSUM accumulation):
  - x^T, ca^T built on-chip via PE transposes (cast to bf16 on PSUM evict)
  - q^T [d_out, q], k^T [d_out, kv] produced directly by the projections
  - scores computed TRANSPOSED: scores^T[kv, q] = k_h @ q_h^T, so softmax's
    kv-reduction becomes a matmul contraction (no cross-partition reduces)
  - head pairs (base partition 0 / 64, K=64) auto row-tile the PE array
  - exp via ScalarE on [128, 2048] PSUM tiles (one per kv-tile, both heads)
  - AV uses v as lhsT augmented with a ones column: out^T[65, q] where row 64
    is the softmax denominator; normalize via partition_broadcast + DVE mul,
    writing attn^T directly in the layout the output projection consumes.
"""
import sys

sys.path.insert(0, "/opt/trn_rl_repo")

from contextlib import ExitStack

import numpy as np

import concourse.bass as bass
import concourse.mybir as mybir
import concourse.tile as tile
from concourse import bacc
from concourse.bass_utils import run_bass_kernel_spmd
from concourse.masks import make_identity

B, SQ, SKV, D, H = 4, 2048, 2048, 512, 8
DH = D // H            # 64 head dim
P = 128
NCORES = 8
SQC = B * SQ // NCORES  # 1024 query rows per core
QT = SQC // P           # 8 q row tiles
KT = SKV // P           # 16 kv tiles
DT = D // P             # 4 embed tiles
HP = H // 2             # 4 head pairs
F32 = mybir.dt.float32
BF16 = mybir.dt.bfloat16
AF = mybir.ActivationFunctionType
EPS = 1e-5
SCALE = DH ** -0.5      # 0.125


def _emit(tc, ctx, io, dbg=None):
    nc = tc.nc
    x_d, ca_d, wq_d, bq_d, wkv_d, bkv_d, wo_d, bo_d, gm_d, bt_d, out_d = io

    const = ctx.enter_context(tc.tile_pool(name="const", bufs=1))
    psA = ctx.enter_context(tc.tile_pool(name="psA", bufs=1, space="PSUM"))
    psB = ctx.enter_context(tc.tile_pool(name="psB", bufs=4, space="PSUM"))

    # ---------- constants / persistent tensors ----------
    ident_f = const.tile([P, P], F32)
    make_identity(nc, ident_f)
    ident_b = const.tile([P, P], BF16)
    nc.vector.tensor_copy(ident_b, ident_f)

    x_sb = const.tile([P, QT, D], F32)          # residual + transpose source
    nc.sync.dma_start(out=x_sb, in_=x_d.rearrange("(t p) d -> p t d", p=P))

    wq_sb = const.tile([P, DT, D], BF16)
    wkv_sb = const.tile([P, DT, 2 * D], BF16)
    wo_sb = const.tile([P, DT, D], BF16)

    bq_sb = const.tile([P, DT], F32)
    nc.sync.dma_start(out=bq_sb, in_=bq_d.rearrange("(t p) -> p t", p=P))
    bk_sb = const.tile([P, DT], F32)
    nc.sync.dma_start(out=bk_sb, in_=bkv_d[0:D].rearrange("(t p) -> p t", p=P))

    def bcast(src_ap, tag, dt=F32):  # replicate a [D] vector over all partitions
        t = const.tile([P, D], dt, tag=tag)
        rep = bass.AP(tensor=src_ap.tensor, offset=src_ap.offset,
                      ap=[[0, P]] + list(src_ap.ap))
        nc.gpsimd.dma_start(out=t, in_=rep)
        return t

    bv_bc = bcast(bkv_d[D:2 * D], "bv_bc", BF16)
    bo_bc = bcast(bo_d[:], "bo_bc")
    gm_bc = bcast(gm_d[:], "gm_bc")
    bt_bc = bcast(bt_d[:], "bt_bc")

    eps_t = const.tile([P, 1], F32)
    nc.vector.memset(eps_t, EPS)

    probs_pool = ctx.enter_context(tc.tile_pool(name="probs", bufs=2))
    work = ctx.enter_context(tc.tile_pool(name="work", bufs=2))
    ep = ctx.enter_context(tc.tile_pool(name="ep", bufs=2))
    p1 = tc.tile_pool(name="p1", bufs=1)
    p1ctx = p1.__enter__()
    ca_scope = tc.tile_pool(name="ca_pool", bufs=2)
    ca_pool = ca_scope.__enter__()
    xT = p1ctx.tile([P, DT, SQC], BF16)
    caT = p1ctx.tile([P, DT, SKV], BF16)
    qT = const.tile([P, DT, SQC], BF16)
    kT = const.tile([P, DT, SKV], BF16)
    v_aug = const.tile([P, KT, H, DH + 1], BF16)
    attnT = const.tile([P, DT, SQC], BF16)

    nc.vector.memset(v_aug[:, :, :, DH:DH + 1], 1.0)  # ones column per head

    # ---------- phase 1: transposes + projections ----------
    def emit_kT_chunk(m, cc):
        acc = psB.tile([P, 512], F32, tag="ps1")
        for kd in range(DT):
            nc.tensor.matmul(acc, wkv_sb[:, kd, m * P:(m + 1) * P],
                             caT[:, kd, cc * 512:(cc + 1) * 512],
                             start=(kd == 0), stop=(kd == DT - 1))
        nc.vector.tensor_scalar_add(kT[:, m, cc * 512:(cc + 1) * 512], acc,
                                    bk_sb[:, m:m + 1])

    def emit_qT_chunk(m, qcx):
        acc = psB.tile([P, 512], F32, tag="ps1")
        for kd in range(DT):
            nc.tensor.matmul(acc, wq_sb[:, kd, m * P:(m + 1) * P],
                             xT[:, kd, qcx * 512:(qcx + 1) * 512],
                             start=(kd == 0), stop=(kd == DT - 1))
        nc.vector.tensor_scalar_add(qT[:, m, qcx * 512:(qcx + 1) * 512], acc,
                                    bq_sb[:, m:m + 1])

    # SWDGE queue order: wkv first (kT critical path), then ca chunks
    nc.gpsimd.dma_start(out=wkv_sb, in_=wkv_d.rearrange("(t p) n -> p t n", p=P))
    for ch in range(KT // 4):
        ca_t = ca_pool.tile([P, 4, D], BF16)
        nc.gpsimd.dma_start(
            out=ca_t, in_=ca_d[ch * 4 * P:(ch + 1) * 4 * P, :]
            .rearrange("(s p) d -> p s d", p=P))
        for si in range(4):
            t = ch * 4 + si
            pt = psB.tile([P, DT, P], BF16, tag="ps1")
            for c in range(DT):
                nc.tensor.transpose(pt[:, c, :], ca_t[:, si, c * P:(c + 1) * P], ident_b)
            nc.vector.tensor_copy(caT[:, :, t * P:(t + 1) * P], pt)
        emit_kT_chunk(0, ch)  # head-pair 0's k^T chunk rides right behind

    # x^T (fp32 in, bf16 out)
    for rt in range(QT):
        pt = psB.tile([P, DT, P], F32, tag="ps1")
        for c in range(DT):
            nc.tensor.transpose(pt[:, c, :], x_sb[:, rt, c * P:(c + 1) * P], ident_f)
        nc.vector.tensor_copy(xT[:, :, rt * P:(rt + 1) * P], pt)

    # fold bo into the residual copy of x (epilogue then skips the bo add)
    for rt in range(QT):
        nc.vector.tensor_add(x_sb[:, rt, :], x_sb[:, rt, :], bo_bc)

    nc.gpsimd.dma_start(out=wq_sb, in_=wq_d.rearrange("(t p) n -> p t n", p=P))
    nc.gpsimd.dma_start(out=wo_sb, in_=wo_d.rearrange("(t p) n -> p t n", p=P))

    from collections import deque
    bg = deque()  # (hp, thunk) background projection work drained inside attention

    def push_proj(m):
        for qcx in range(SQC // 512):
            bg.append((m, lambda m=m, q=qcx: emit_qT_chunk(m, q)))
        for cc in range(SKV // 512):
            bg.append((m, lambda m=m, c=cc: emit_kT_chunk(m, c)))

    def drain_for(hp):
        while bg and bg[0][0] <= hp:
            bg.popleft()[1]()

    def emit_v(t):
        acc = psB.tile([P, 512], F32, tag="ps1")
        for kd in range(DT):
            nc.tensor.matmul(acc, caT[:, kd, t * P:(t + 1) * P],
                             wkv_sb[:, kd, D:2 * D],
                             start=(kd == 0), stop=(kd == DT - 1))
        nc.vector.tensor_add(
            v_aug[:, t, :, 0:DH],
            acc.rearrange("p (h d) -> p h d", h=H),
            bv_bc.rearrange("p (h d) -> p h d", h=H))

    # ---------- phase 2+3: attention with interleaved projections ----------

    def attention(qc, hp, inline_v=False):
        drain_for(hp)
        q0 = qc * 512
        h0, h1 = 2 * hp, 2 * hp + 1
        probs = probs_pool.tile([P, KT, 2, 512], BF16)
        av0 = psB.tile([P, 512], F32, tag="ps1")
        av1 = psB.tile([P, 512], F32, tag="ps1")

        def emit_av(t):
            nc.tensor.matmul(av0[0:DH + 1, :], v_aug[:, t, h0, :],
                             probs[:, t, 0, :],
                             start=(t == 0), stop=(t == KT - 1))
            nc.tensor.matmul(av1[0:DH + 1, :], v_aug[:, t, h1, :],
                             probs[:, t, 1, :],
                             start=(t == 0), stop=(t == KT - 1))

        for t in range(KT):
            if inline_v:
                emit_v(t)  # v[t] ready before emit_av(t) next iteration
            ss = psA.tile([P, 2, 512], F32)
            nc.tensor.matmul(ss[:, 0, :],
                             kT[0:DH, hp, t * P:(t + 1) * P],
                             qT[0:DH, hp, q0:q0 + 512])
            nc.tensor.matmul(ss[:, 1, :],
                             kT[DH:P, hp, t * P:(t + 1) * P],
                             qT[DH:P, hp, q0:q0 + 512])
            nc.scalar.activation(probs[:, t, :, :], ss, AF.Exp, scale=SCALE)
        for t in range(KT):
            emit_av(t)
        for hl, av in ((0, av0), (1, av1)):
            den = work.tile([DH + 1, 512], F32, tag="den")
            dbc = work.tile([DH, 512], F32, tag="dbc")
            nc.vector.reciprocal(den[DH:DH + 1, :], av[DH:DH + 1, :])
            # partition_broadcast only reads partition 0 on HW: hop 64->0
            nc.gpsimd.tensor_copy(den[0:1, :], den[DH:DH + 1, :])
            nc.gpsimd.partition_broadcast(dbc, den[0:1, :])
            nc.vector.tensor_mul(
                attnT[hl * DH:(hl + 1) * DH, hp, q0:q0 + 512],
                av[0:DH, :], dbc)

    def epilogue(qc):
        for j in range(4):
            qt = qc * 4 + j
            acc = psB.tile([P, 512], F32, tag="ps1")
            for kd in range(DT):
                nc.tensor.matmul(acc, attnT[:, kd, qt * P:(qt + 1) * P],
                                 wo_sb[:, kd], start=(kd == 0), stop=(kd == DT - 1))
            t1 = ep.tile([P, D], F32, tag="t1")
            nc.vector.tensor_add(t1, acc, x_sb[:, qt, :])  # x_sb carries +bo
            stats = ep.tile([P, 6], F32, tag="stats")
            nc.vector.bn_stats(stats, t1)
            mv = ep.tile([P, 2], F32, tag="mv")
            nc.vector.bn_aggr(mv, stats)
            rstd = ep.tile([P, 1], F32, tag="rstd")
            nc.scalar.activation(rstd, mv[:, 1:2], AF.Sqrt, bias=eps_t)
            nc.vector.reciprocal(rstd, rstd)
            nc.vector.tensor_scalar(t1, t1, scalar1=mv[:, 0:1], scalar2=rstd,
                                    op0=mybir.AluOpType.subtract,
                                    op1=mybir.AluOpType.mult)
            nc.vector.tensor_mul(t1, t1, gm_bc)
            nc.gpsimd.tensor_add(t1, t1, bt_bc)
            nc.sync.dma_start(out_d[qt * P:(qt + 1) * P, :], t1)

    # minimal prefix for (qc0, hp0), then fill PE slack under ACT's exps
    for qcx in range(SQC // 512):
        emit_qT_chunk(0, qcx)
    for m in range(1, DT):
        push_proj(m)
    attention(0, 0, inline_v=True)
    for hp in range(1, HP):
        attention(0, hp)
    ca_scope.__exit__(None, None, None)
    p1.__exit__(None, None, None)
    epilogue(0)
    for hp in range(HP):
        attention(1, hp)
    epilogue(1)

    if dbg is not None:
        nc.gpsimd.dma_start(out=dbg["qT"], in_=qT)
        nc.gpsimd.dma_start(out=dbg["kT"], in_=kT)
        nc.gpsimd.dma_start(out=dbg["v_aug"], in_=v_aug)
        nc.gpsimd.dma_start(out=dbg["attnT"], in_=attnT)


def _build(debug=False):
    nc = bacc.Bacc("TRN2", target_bir_lowering=False, debug=False,
                   num_devices=NCORES)
    io = (
        nc.dram_tensor("x", [SQC, D], F32, kind="ExternalInput").ap(),
        nc.dram_tensor("ca", [SKV, D], F32, kind="ExternalInput").ap(),
        nc.dram_tensor("wq", [D, D], F32, kind="ExternalInput").ap(),
        nc.dram_tensor("bq", [D], F32, kind="ExternalInput").ap(),
        nc.dram_tensor("wkv", [D, 2 * D], F32, kind="ExternalInput").ap(),
        nc.dram_tensor("bkv", [2 * D], F32, kind="ExternalInput").ap(),
        nc.dram_tensor("wo", [D, D], F32, kind="ExternalInput").ap(),
        nc.dram_tensor("bo", [D], F32, kind="ExternalInput").ap(),
        nc.dram_tensor("gamma", [D], F32, kind="ExternalInput").ap(),
        nc.dram_tensor("beta", [D], F32, kind="ExternalInput").ap(),
        nc.dram_tensor("out", [SQC, D], F32, kind="ExternalOutput").ap(),
    )
    dbg = None
    if debug:
        dbg = {
            "qT": nc.dram_tensor("d_qT", [P, DT, SQC], F32, kind="ExternalOutput").ap(),
            "kT": nc.dram_tensor("d_kT", [P, DT, SKV], F32, kind="ExternalOutput").ap(),
            "v_aug": nc.dram_tensor("d_v", [P, KT, H, DH + 1], F32, kind="ExternalOutput").ap(),
            "attnT": nc.dram_tensor("d_aT", [P, DT, SQC], F32, kind="ExternalOutput").ap(),
        }
    with tile.TileContext(nc) as tc, ExitStack() as ctx:
        _emit(tc, ctx, io, dbg)
    nc.compile()
    return nc


_CACHE = {}


def _get_nc():
    if "nc" not in _CACHE:
        _CACHE["nc"] = _build()
    return _CACHE["nc"]


def kernel(layer_input, cross_attn_input, Wq, bq, Wkv, bkv, Wo, bo, gamma,
           beta, trace=False):
    f32 = np.float32
    layer_input = np.ascontiguousarray(layer_input, dtype=f32)
    cross_attn_input = np.ascontiguousarray(cross_attn_input, dtype=f32)
    shared = {
        "wq": np.ascontiguousarray(Wq, f32),
        "bq": np.ascontiguousarray(bq, f32),
        "wkv": np.ascontiguousarray(Wkv, f32),
        "bkv": np.ascontiguousarray(bkv, f32),
        "wo": np.ascontiguousarray(Wo, f32),
        "bo": np.ascontiguousarray(bo, f32),
        "gamma": np.ascontiguousarray(gamma, f32),
        "beta": np.ascontiguousarray(beta, f32),
    }
    in_maps = []
    for c in range(NCORES):
        b, r = c // 2, c % 2
        in_maps.append({
            "x": np.ascontiguousarray(layer_input[b, r * SQC:(r + 1) * SQC, :]),
            "ca": np.ascontiguousarray(cross_attn_input[b]),
            **shared,
        })
    nc = _get_nc()
    res = run_bass_kernel_spmd(nc, in_maps, core_ids=list(range(NCORES)),
                               trace=trace)
    out = np.empty((B, SQ, D), np.float32)
    for c in range(NCORES):
        b, r = c // 2, c % 2
        out[b, r * SQC:(r + 1) * SQC, :] = res.results[c]["out"]
    if trace:
        return out, res
    return out



# revision 15
# speedup vs baseline: 1.3483x; 1.3483x over previous
"""Trainium2 Bass kernel for CrossAttention + residual + LayerNorm.

Problem: B=4, Sq=Skv=2048, D=512, H=8 heads (dh=64), fp32 I/O.

Sharding (8 cores, no collectives): core c handles batch b=c//2 and query-row
half r=c%2 (1024 q rows). Each core loads its x slice [1024,512], the full
cross-attn input for its batch [2048,512], and all weights; computes
q/k/v projections, per-head softmax(q k^T / 8) @ v, output projection,
residual add and layernorm for its 1024 rows. Host reassembles.

Layout strategy (all matmul operands bf16, fp32 PSUM accumulation):
  - x^T, ca^T built on-chip via PE transposes (cast to bf16 on PSUM evict)
  - q^T [d_out, q], k^T [d_out, kv] produced directly by the projections
  - scores computed TRANSPOSED: scores^T[kv, q] = k_h @ q_h^T, so softmax's
    kv-reduction becomes a matmul contraction (no cross-partition reduces)
  - head pairs (base partition 0 / 64, K=64) auto row-tile the PE array
  - exp via ScalarE on double-buffered [128, 2x512] PSUM tiles; the AV
    matmuls for tile t-1 are interleaved right behind scores tile t so the
    PE never idles during the exp stream (keeps the HAM clock gate warm)
  - AV uses v as lhsT augmented with a ones column: out^T[65, q] where row 64
    is the softmax denominator. Unnormalized av is copied to attn^T (DVE);
    denominators are gathered to an [8, 512] tile, one reciprocal per
    q-chunk, broadcast across partitions via a PE outer-product with a
    one-hot head-selector, then 4 in-place DVE muls normalize attn^T.
  - LayerNorm rstd = exp(-0.5*ln(var+eps)) so the whole kernel uses one
    ACT table set (no exp<->sqrt table reloads)
"""
import sys

sys.path.insert(0, "/opt/trn_rl_repo")

from collections import deque
from contextlib import ExitStack

import numpy as np

import concourse.bass as bass
import concourse.mybir as mybir
import concourse.tile as tile
from concourse import bacc
from concourse.bass_utils import run_bass_kernel_spmd
from concourse.masks import make_identity

B, SQ, SKV, D, H = 4, 2048, 2048, 512, 8
DH = D // H            # 64 head dim
P = 128
NCORES = 8
SQC = B * SQ // NCORES  # 1024 query rows per core
QT = SQC // P           # 8 q row tiles
KT = SKV // P           # 16 kv tiles
DT = D // P             # 4 embed tiles
HP = H // 2             # 4 head pairs
F32 = mybir.dt.float32
BF16 = mybir.dt.bfloat16
AF = mybir.ActivationFunctionType
EPS = 1e-5
SCALE = DH ** -0.5      # 0.125
WARMUP_MM = 40          # PE spin during initial DMA wait (HAM un-throttle)


def _emit(tc, ctx, io):
    nc = tc.nc
    x_d, ca_d, wq_d, bq_d, wkv_d, bkv_d, wo_d, bo_d, gm_d, bt_d, out_d = io

    const = ctx.enter_context(tc.tile_pool(name="const", bufs=1))
    psA = ctx.enter_context(tc.tile_pool(name="psA", bufs=2, space="PSUM"))
    psB = ctx.enter_context(tc.tile_pool(name="psB", bufs=4, space="PSUM"))

    # ---------- constants ----------
    ident_f = const.tile([P, P], F32)
    make_identity(nc, ident_f)
    ident_b = const.tile([P, P], BF16)
    nc.vector.tensor_copy(ident_b, ident_f)

    # ---------- critical-path DMAs first ----------
    # tiny per-channel biases (needed by the first kT/qT chunks)
    bq_sb = const.tile([P, DT], F32)
    nc.gpsimd.dma_start(out=bq_sb, in_=bq_d.rearrange("(t p) -> p t", p=P))
    bk_sb = const.tile([P, DT], F32)
    nc.gpsimd.dma_start(out=bk_sb, in_=bkv_d[0:D].rearrange("(t p) -> p t", p=P))
    def load_row(src_ap, tag, dt):
        t = const.tile([1, D], dt, tag=tag, name=tag)
        rep = bass.AP(tensor=src_ap.tensor, offset=src_ap.offset,
                      ap=[[0, 1]] + list(src_ap.ap))
        nc.gpsimd.dma_start(out=t, in_=rep)
        return t

    # all casting DMAs (f32->bf16) must ride the gpsimd DGE queue; order by
    # criticality: wkv, ca chunks (kT path), wq (qT path), wo, then vectors
    wkv_sb = const.tile([P, DT, 2 * D], BF16)
    nc.gpsimd.dma_start(out=wkv_sb, in_=wkv_d.rearrange("(t p) n -> p t n", p=P))
    ca_ts = []
    ca_pool = ctx.enter_context(tc.tile_pool(name="ca_pool", bufs=4))
    wq_sb = const.tile([P, DT, D], BF16)
    wo_sb = const.tile([P, DT, D], BF16)
    for ch in range(KT // 4):
        ca_t = ca_pool.tile([P, 4, D], BF16, tag="ca", name=f"ca{ch}")
        nc.gpsimd.dma_start(
            out=ca_t, in_=ca_d[ch * 4 * P:(ch + 1) * 4 * P, :]
            .rearrange("(s p) d -> p s d", p=P))
        ca_ts.append(ca_t)
        if ch == 0:
            nc.gpsimd.dma_start(
                out=wq_sb, in_=wq_d.rearrange("(t p) n -> p t n", p=P))
    nc.gpsimd.dma_start(out=wo_sb, in_=wo_d.rearrange("(t p) n -> p t n", p=P))
    bv_row = load_row(bkv_d[D:2 * D], "bv_row", BF16)

    # epilogue vectors: load one row, replicate on-chip (gpsimd is idle)
    def bcast(src_ap, tag, dt=F32):
        row = load_row(src_ap, tag + "_row", dt)
        t = const.tile([P, D], dt, tag=tag, name=tag)
        nc.gpsimd.partition_broadcast(t, row)
        return t

    bv_bc = const.tile([P, D], BF16)
    nc.gpsimd.partition_broadcast(bv_bc, bv_row)
    bo_bc = bcast(bo_d[:], "bo_bc")
    gm_bc = bcast(gm_d[:], "gm_bc")
    bt_bc = bcast(bt_d[:], "bt_bc")

    eps_t = const.tile([P, 1], F32)
    nc.vector.memset(eps_t, EPS)

    # head selector for the denominator broadcast (NEFF-embedded const):
    # within embed-tile kd, partitions [0,64) hold head 2kd and [64,128) hold
    # head 2kd+1; head h's denominator is gathered at partition 32*(h%4) of
    # den_lo (h<4) / den_hi (h>=4).  sel8[32*(h%4), kd*128+j] = 1 iff
    # head(kd, j) == h, so  sel8[:, kd].T @ rec  broadcasts 1/den per channel.
    sel_np = np.zeros((P, D), np.float32)
    for kd in range(DT):
        for j in range(P):
            h = 2 * kd + j // DH
            sel_np[32 * (h % 4), kd * P + j] = 1.0
    sel_d = nc.inline_tensor(sel_np, name="sel8c")
    sel8 = const.tile([P, D], BF16)
    nc.gpsimd.dma_start(out=sel8, in_=sel_d.ap())
    den_lo = const.tile([P, 512], F32)
    nc.vector.memset(den_lo, 1.0)   # unused rows stay finite for reciprocal
    den_hi = const.tile([P, 512], F32)
    nc.vector.memset(den_hi, 1.0)

    # ---------- PE warmup: keep the HAM clock gate from idling ----------
    for w in range(WARMUP_MM // 4):
        wt = psB.tile([P, 512], BF16, tag="ps1", name=f"warm{w}")
        for c in range(4):
            nc.tensor.transpose(wt[:, c * P:(c + 1) * P], ident_b, ident_b)

    # ---------- persistent tensors ----------
    probs_pool = ctx.enter_context(tc.tile_pool(name="probs", bufs=2))
    work = ctx.enter_context(tc.tile_pool(name="work", bufs=2))
    ep = ctx.enter_context(tc.tile_pool(name="ep", bufs=2))
    p1ctx = ctx.enter_context(tc.tile_pool(name="p1", bufs=1))
    xT = p1ctx.tile([P, DT, SQC], BF16)
    caT = p1ctx.tile([P, DT, SKV], BF16)
    x_sb = const.tile([P, QT, D], F32)          # residual + transpose source
    qT = const.tile([P, DT, SQC], BF16)
    kT = const.tile([P, DT, SKV], BF16)
    v_aug = const.tile([P, KT, H, DH + 1], BF16)
    attnT = const.tile([P, DT, SQC], BF16)

    nc.vector.memset(v_aug[:, :, :, DH:DH + 1], 1.0)  # ones column per head

    # ---------- phase 1: transposes + projections ----------
    def emit_kT_chunk(m, cc):
        acc = psB.tile([P, 512], F32, tag="ps1")
        for kd in range(DT):
            nc.tensor.matmul(acc, wkv_sb[:, kd, m * P:(m + 1) * P],
                             caT[:, kd, cc * 512:(cc + 1) * 512],
                             start=(kd == 0), stop=(kd == DT - 1))
        nc.vector.tensor_scalar_add(kT[:, m, cc * 512:(cc + 1) * 512], acc,
                                    bk_sb[:, m:m + 1])

    def emit_qT_chunk(m, qcx):
        acc = psB.tile([P, 512], F32, tag="ps1")
        for kd in range(DT):
            nc.tensor.matmul(acc, wq_sb[:, kd, m * P:(m + 1) * P],
                             xT[:, kd, qcx * 512:(qcx + 1) * 512],
                             start=(kd == 0), stop=(kd == DT - 1))
        nc.vector.tensor_scalar_add(qT[:, m, qcx * 512:(qcx + 1) * 512], acc,
                                    bq_sb[:, m:m + 1])

    def emit_x_half(xh):
        # transpose x rows [xh*512, xh*512+512) and fold +bo into the residual
        for rt in range(4 * xh, 4 * xh + 4):
            pt = psB.tile([P, DT, P], F32, tag="ps1")
            for c in range(DT):
                nc.tensor.transpose(pt[:, c, :], x_sb[:, rt, c * P:(c + 1) * P],
                                    ident_f)
            nc.vector.tensor_copy(xT[:, :, rt * P:(rt + 1) * P], pt)
            nc.vector.tensor_add(x_sb[:, rt, :], x_sb[:, rt, :], bo_bc)

    # x (no cast) streams on the sync DGE queue, in two halves
    nc.sync.dma_start(out=x_sb[:, 0:4, :],
                      in_=x_d[0:512, :].rearrange("(t p) d -> p t d", p=P))
    nc.sync.dma_start(out=x_sb[:, 4:8, :],
                      in_=x_d[512:1024, :].rearrange("(t p) d -> p t d", p=P))

    for ch in range(KT // 4):
        ca_t = ca_ts[ch]
        for si in range(4):
            t = ch * 4 + si
            pt = psB.tile([P, DT, P], BF16, tag="ps1")
            for c in range(DT):
                nc.tensor.transpose(pt[:, c, :], ca_t[:, si, c * P:(c + 1) * P],
                                    ident_b)
            nc.vector.tensor_copy(caT[:, :, t * P:(t + 1) * P], pt)
        emit_kT_chunk(0, ch)  # head-pair 0's k^T chunk rides right behind
        if ch == 1:
            emit_x_half(0)
            emit_qT_chunk(0, 0)

    # background work drained a-thunk-per-tile inside the attention loops.
    # keys are attention-call ordinals (qc*HP + hp) the thunk must precede.
    bg = deque()  # ordered by key

    def push_proj(m):
        bg.append((m, lambda m=m: emit_qT_chunk(m, 0)))
        for cc in range(SKV // 512):
            bg.append((m, lambda m=m, c=cc: emit_kT_chunk(m, c)))

    def drain_for(key):
        while bg and bg[0][0] <= key:
            bg.popleft()[1]()

    def drain_one():
        if bg:
            bg.popleft()[1]()

    def emit_v(t):
        acc = psB.tile([P, 512], F32, tag="ps1")
        for kd in range(DT):
            nc.tensor.matmul(acc, caT[:, kd, t * P:(t + 1) * P],
                             wkv_sb[:, kd, D:2 * D],
                             start=(kd == 0), stop=(kd == DT - 1))
        nc.vector.tensor_add(
            v_aug[:, t, :, 0:DH],
            acc.rearrange("p (h d) -> p h d", h=H),
            bv_bc.rearrange("p (h d) -> p h d", h=H))

    # ---------- phase 2+3: attention with interleaved projections ----------

    def attention(qc, hp, inline_v=False):
        drain_for(qc * HP + hp)
        q0 = qc * 512
        h0, h1 = 2 * hp, 2 * hp + 1
        pairs = []  # rotating per-t-pair probs tiles [P, 2(t), 2(hl), 512]
        av0 = psB.tile([P, 512], F32, tag="ps1")
        av1 = psB.tile([P, 512], F32, tag="ps1")

        def emit_av(t):
            pt = pairs[t // 2][:, t % 2]
            nc.tensor.matmul(av0[0:DH + 1, :], v_aug[:, t, h0, :], pt[:, 0, :],
                             start=(t == 0), stop=(t == KT - 1))
            nc.tensor.matmul(av1[0:DH + 1, :], v_aug[:, t, h1, :], pt[:, 1, :],
                             start=(t == 0), stop=(t == KT - 1))

        for t in range(KT):
            if inline_v:
                emit_v(t)  # v[t] ready before emit_av(t) next iteration
            else:
                drain_one()
            if t % 2 == 0:
                pairs.append(probs_pool.tile([P, 2, 2, 512], BF16, tag="probs",
                                             bufs=3, name="probs"))
            ss = psA.tile([P, 2, 512], F32)
            nc.tensor.matmul(ss[:, 0, :],
                             kT[0:DH, hp, t * P:(t + 1) * P],
                             qT[0:DH, hp, q0:q0 + 512])
            nc.tensor.matmul(ss[:, 1, :],
                             kT[DH:P, hp, t * P:(t + 1) * P],
                             qT[DH:P, hp, q0:q0 + 512])
            nc.scalar.activation(pairs[-1][:, t % 2, :, :], ss, AF.Exp,
                                 scale=SCALE)
            if t > 0:
                emit_av(t - 1)
        emit_av(KT - 1)
        # unnormalized av -> attnT; denominator rows gathered (32-aligned)
        for hl, av in ((0, av0), (1, av1)):
            h = 2 * hp + hl
            nc.vector.tensor_copy(attnT[hl * DH:(hl + 1) * DH, hp, q0:q0 + 512],
                                  av[0:DH, :])
            dst = den_lo if h < 4 else den_hi
            r = 32 * (h % 4)
            nc.vector.tensor_copy(dst[r:r + 1, :], av[DH:DH + 1, :])

    def finish_qc(qc):
        # one reciprocal per half, PE broadcast, 4 in-place muls
        q0 = qc * 512
        for half, den_t in ((0, den_lo), (1, den_hi)):
            rec_f = work.tile([P, 512], F32, tag=f"rec{half}", name=f"rec{half}")
            nc.vector.reciprocal_approx_fast(rec_f, den_t)
            rec_b = work.tile([P, 512], BF16, tag=f"recb{half}",
                              name=f"recb{half}")
            nc.vector.tensor_copy(rec_b, rec_f)
            for kd in (2 * half, 2 * half + 1):
                bc = psB.tile([P, 512], F32, tag="ps1")
                nc.tensor.matmul(bc, sel8[:, kd * P:(kd + 1) * P], rec_b,
                                 start=True, stop=True)
                nc.vector.tensor_mul(attnT[:, kd, q0:q0 + 512],
                                     attnT[:, kd, q0:q0 + 512], bc)

    def epi_chunk(qt):
        acc = psB.tile([P, 512], F32, tag="ps1")
        for kd in range(DT):
            nc.tensor.matmul(acc, attnT[:, kd, qt * P:(qt + 1) * P],
                             wo_sb[:, kd], start=(kd == 0), stop=(kd == DT - 1))
        t1 = ep.tile([P, D], F32, tag="t1")
        nc.vector.tensor_add(t1, acc, x_sb[:, qt, :])  # x_sb carries +bo
        stats = ep.tile([P, 6], F32, tag="stats")
        nc.vector.bn_stats(stats, t1)
        mv = ep.tile([P, 2], F32, tag="mv")
        nc.vector.bn_aggr(mv, stats)
        lnv = ep.tile([P, 1], F32, tag="lnv")
        nc.scalar.activation(lnv, mv[:, 1:2], AF.Ln, bias=eps_t)
        rstd = ep.tile([P, 1], F32, tag="rstd")
        nc.scalar.activation(rstd, lnv, AF.Exp, scale=-0.5)
        nc.vector.tensor_scalar(t1, t1, scalar1=mv[:, 0:1], scalar2=rstd,
                                op0=mybir.AluOpType.subtract,
                                op1=mybir.AluOpType.mult)
        nc.vector.tensor_mul(t1, t1, gm_bc)
        nc.gpsimd.tensor_add(t1, t1, bt_bc)
        nc.sync.dma_start(out_d[qt * P:(qt + 1) * P, :], t1)

    # proj thunks for head-pairs 1-3 (key m: needed before attention(0, m));
    # the second x half + all qc=1 qT chunks keyed HP+m (before attention(1,m))
    for m in range(1, DT):
        push_proj(m)
    bg.append((HP, lambda: (emit_x_half(1), emit_qT_chunk(0, 1))))
    for m in range(1, DT):
        bg.append((HP + m, lambda m=m: emit_qT_chunk(m, 1)))
    bg = deque(sorted(bg, key=lambda kv: kv[0]))

    attention(0, 0, inline_v=True)
    for hp in range(1, HP):
        attention(0, hp)
    finish_qc(0)
    attention(1, 0)
    for j in range(4):
        bg.append((99, lambda q=j: epi_chunk(q)))
    for hp in range(1, HP):
        attention(1, hp)
    drain_for(99)
    finish_qc(1)
    for j in range(4, 8):
        epi_chunk(j)


def _build():
    nc = bacc.Bacc("TRN2", target_bir_lowering=False, debug=False,
                   num_devices=NCORES)
    io = (
        nc.dram_tensor("x", [SQC, D], F32, kind="ExternalInput").ap(),
        nc.dram_tensor("ca", [SKV, D], F32, kind="ExternalInput").ap(),
        nc.dram_tensor("wq", [D, D], F32, kind="ExternalInput").ap(),
        nc.dram_tensor("bq", [D], F32, kind="ExternalInput").ap(),
        nc.dram_tensor("wkv", [D, 2 * D], F32, kind="ExternalInput").ap(),
        nc.dram_tensor("bkv", [2 * D], F32, kind="ExternalInput").ap(),
        nc.dram_tensor("wo", [D, D], F32, kind="ExternalInput").ap(),
        nc.dram_tensor("bo", [D], F32, kind="ExternalInput").ap(),
        nc.dram_tensor("gamma", [D], F32, kind="ExternalInput").ap(),
        nc.dram_tensor("beta", [D], F32, kind="ExternalInput").ap(),
        nc.dram_tensor("out", [SQC, D], F32, kind="ExternalOutput").ap(),
    )
    with tile.TileContext(nc) as tc, ExitStack() as ctx:
        _emit(tc, ctx, io)
    nc.compile()
    return nc


_CACHE = {}


def _get_nc():
    if "nc" not in _CACHE:
        _CACHE["nc"] = _build()
    return _CACHE["nc"]


def kernel(layer_input, cross_attn_input, Wq, bq, Wkv, bkv, Wo, bo, gamma,
           beta, trace=False):
    f32 = np.float32
    layer_input = np.ascontiguousarray(layer_input, dtype=f32)
    cross_attn_input = np.ascontiguousarray(cross_attn_input, dtype=f32)
    shared = {
        "wq": np.ascontiguousarray(Wq, f32),
        "bq": np.ascontiguousarray(bq, f32),
        "wkv": np.ascontiguousarray(Wkv, f32),
        "bkv": np.ascontiguousarray(bkv, f32),
        "wo": np.ascontiguousarray(Wo, f32),
        "bo": np.ascontiguousarray(bo, f32),
        "gamma": np.ascontiguousarray(gamma, f32),
        "beta": np.ascontiguousarray(beta, f32),
    }
    in_maps = []
    for c in range(NCORES):
        b, r = c // 2, c % 2
        in_maps.append({
            "x": np.ascontiguousarray(layer_input[b, r * SQC:(r + 1) * SQC, :]),
            "ca": np.ascontiguousarray(cross_attn_input[b]),
            **shared,
        })
    nc = _get_nc()
    res = run_bass_kernel_spmd(nc, in_maps, core_ids=list(range(NCORES)),
                               trace=trace)
    out = np.empty((B, SQ, D), np.float32)
    for c in range(NCORES):
        b, r = c // 2, c % 2
        out[b, r * SQC:(r + 1) * SQC, :] = res.results[c]["out"]
    if trace:
        return out, res
    return out
